# revision 6
# baseline (speedup 1.0000x reference)
"""Causal depthwise-conv self-attention kernel for Trainium2 (8 NeuronCores).

Math: out[b,t,d] = sum_i sum_k X[b,t-i,k] * W[i*D+d,k]   (i in 0..kW-1, zero for t<i)

Algorithm: Winograd F(4,4) over the time axis with points {0,1,-1,2,-2,1/2,inf}.
Each tile of 4 outputs needs 7 transform-point products instead of 16 tap-MACs,
cutting PE work to 7/16. Host applies the input transform B^T (7 points per
4-wide tile, exact fp32, cast fp16). The weight side uploads the fp16 BASIS
{g0, g3, b1=g0+g1+g2+g3, b2=g0-g1+g2-g3} (g = flipped taps): these are exactly
the G-transformed weights for points {0, inf, 1, -1}, so the first four
processed points need no device transform; points {2,-2,1/2} are derived from
the basis on the vector engine while those matmuls are still far away. The
device runs the 7 per-point (couts x cins) matmuls with fp32 PSUM
accumulation; host applies the 4x7 inverse transform A^T in fp32.

Sharding: 8 cores = 2 batches x 2 T-halves x 2 cout-halves. Per core:
X~ [7,128,8,512] fp16 (7.3 MB) + basis [4,8,128,512] fp16 (4.2 MB) in,
M [7,128,4,512] fp16 (3.7 MB) out, vs the 224-matmul (~48 us) PE floor.
DMA rings carry chunks in first-need order so the PE streams without stalls.
"""

import numpy as np

import concourse.bacc as bacc
import concourse.mybir as mybir
import concourse.tile as tile
from concourse.bass_utils import run_bass_kernel_spmd

# bass_utils imports antenv.axon_hooks when BASS_TRACE is set; that module is
# absent from this image. Provide a no-op stand-in so tracing degrades
# gracefully instead of crashing the run.
try:
    import antenv.axon_hooks  # noqa: F401
except ImportError:
    import sys
    import types

    import antenv

    _hooks = types.ModuleType("antenv.axon_hooks")
    _hooks._h = None
    _hooks.set_axon_ntff_profile_hook = lambda h: setattr(_hooks, "_h", h)
    _hooks.get_axon_ntff_profile_hook = lambda: _hooks._h
    sys.modules["antenv.axon_hooks"] = _hooks
    antenv.axon_hooks = _hooks

BSZ, T, D, KW = 2, 4096, 1024, 4
NCORES = 8
NPT = 7            # Winograd transform points for F(4,4)
UT = T // 4        # 4-wide output tiles = 1024
UTH = UT // 2      # tiles per T-half core = 512
KC = D // 128      # contraction chunks = 8
COH = D // 2       # output channels per cout-half core = 512
CS = COH // 128    # cout subtiles per core = 4
WARMUP_MMS = 10    # PE busy-burst during initial DMA (flips HAM to 8/8)
PT_ORDER = [0, 6, 1, 2, 3, 4, 5]   # basis points first, derived points last
# wr basis slot per point (pts 3,4,5 live in wt3_sb slots 0,1,2)
WR_SLOT = {0: 0, 6: 1, 1: 2, 2: 3}

_last_results = None   # test harness peeks at this for profiling info
_nc_cache = None       # compiled program reused across kernel() calls


def _build_transforms():
    points = [0.0, 1.0, -1.0, 2.0, -2.0, 0.5]   # 6 finite points + infinity
    V = np.zeros((7, 7))
    for k in range(7):
        for p, a in enumerate(points):
            V[k, p] = a ** k
    V[6, 6] = 1.0
    A = np.zeros((7, 4))
    for p, a in enumerate(points):
        for s in range(4):
            A[p, s] = a ** s
    A[6, 3] = 1.0
    BT = np.linalg.inv(V)
    return A.astype(np.float32), BT.astype(np.float32)


A_M, BT_M = _build_transforms()


def _build_nc():
    nc = bacc.Bacc(trn_type="TRN2", enable_partition_id=False)
    # X~ split into per-(pt, kc-pair) 262KB chunks for fine-grained arrival
    xt = nc.dram_tensor("xt", [NPT, KC // 2, 128, 2, UTH], mybir.dt.float16,
                        kind="ExternalInput")
    # weight basis {g0, g3, b1, b2}, per-(slot, kc) 131KB chunks
    wr = nc.dram_tensor("wr", [KW, KC, 128, COH], mybir.dt.float16,
                        kind="ExternalInput")
    mout = nc.dram_tensor("mout", [NPT, 128, CS, UTH], mybir.dt.float16,
                          kind="ExternalOutput")

    with tile.TileContext(nc) as tc:
        with (
            tc.tile_pool(name="xpool", bufs=1) as xpool,
            tc.tile_pool(name="wpool", bufs=1) as wpool,
            tc.tile_pool(name="tpool", bufs=2) as tpool,
            tc.tile_pool(name="opool", bufs=4) as opool,
            tc.tile_pool(name="psum", bufs=8, space="PSUM") as psum_pool,
        ):
            xt_sb = xpool.tile([128, NPT, KC, UTH], mybir.dt.float16)
            wr_sb = wpool.tile([128, KW, KC, COH], mybir.dt.float16)
            wt3_sb = wpool.tile([128, 3, KC, COH], mybir.dt.float16,
                                name="wt3")
            dummy = wpool.tile([128, 512], mybir.dt.float16, name="dummy")
            nc.gpsimd.memset(dummy[:].bitcast(mybir.dt.float32), 0.0)

            # DMA issue order == first-need order; the two HWDGE rings drain
            # round-robin at ~equal byte rate, so splitting the global
            # first-need sequence evenly across them realizes it on arrival.
            # sync:   xt0, xt6, xt1, b1[kc0-3], xt2, xt3, xt4, xt5
            # scalar: g0, g3, b1[kc4-7], b2            (+ mout later)
            def xt_chunks(pt):
                for i in range(KC // 2):
                    nc.sync.dma_start(xt_sb[:, pt, 2 * i:2 * i + 2, :],
                                      xt[pt, i])
            def wr_chunks(j, kcs):
                for kc in kcs:
                    nc.scalar.dma_start(wr_sb[:, j, kc], wr[j, kc])

            xt_chunks(0)
            xt_chunks(6)
            xt_chunks(1)
            for kc in range(4):
                nc.sync.dma_start(wr_sb[:, 2, kc], wr[2, kc])
            for pt in (2, 3, 4, 5):
                xt_chunks(pt)
            wr_chunks(0, range(KC))
            wr_chunks(1, range(KC))
            wr_chunks(2, range(4, KC))
            wr_chunks(3, range(KC))

            # HAM warmup: keep PE busy while the first DMAs land.
            ps_w = psum_pool.tile([128, 512], mybir.dt.float32,
                                  name="ps_warm", tag="ps")
            for w in range(WARMUP_MMS):
                nc.tensor.matmul(ps_w[:], dummy[:, :128], dummy[:],
                                 start=True, stop=True, skip_group_check=True)

            # Derive the weights of points {2,-2,1/2} from the basis
            # (vector engine only; consumed by the last three point groups):
            #   W3 = 3 b1 + b2 - 3 g0 + 6 g3
            #   W4 = b1 + 3 b2 - 3 g0 - 6 g3
            #   W5 = 0.375 (b1 - (1/3) b2 - g3 + 2 g0)
            mult, add = mybir.AluOpType.mult, mybir.AluOpType.add
            stt = nc.vector.scalar_tensor_tensor
            for kc in range(KC):
                g0, g3 = wr_sb[:, 0, kc], wr_sb[:, 1, kc]
                b1, b2 = wr_sb[:, 2, kc], wr_sb[:, 3, kc]
                tmp = {}
                for tg in ("c1", "c2", "d1", "d2", "f1", "f2", "g1"):
                    tmp[tg] = tpool.tile([128, COH], mybir.dt.float16,
                                         name=f"{tg}_{kc}", tag=tg)
                stt(tmp["c1"][:], b1, 3.0, b2, mult, add)
                stt(tmp["c2"][:], g0, -3.0, tmp["c1"][:], mult, add)
                stt(wt3_sb[:, 0, kc], g3, 6.0, tmp["c2"][:], mult, add)
                stt(tmp["d1"][:], b2, 3.0, b1, mult, add)
                stt(tmp["d2"][:], g0, -3.0, tmp["d1"][:], mult, add)
                stt(wt3_sb[:, 1, kc], g3, -6.0, tmp["d2"][:], mult, add)
                stt(tmp["f1"][:], b2, -1.0 / 3.0, b1, mult, add)
                nc.vector.tensor_sub(tmp["f2"][:], tmp["f1"][:], g3)
                stt(tmp["g1"][:], g0, 2.0, tmp["f2"][:], mult, add)
                nc.vector.tensor_scalar(wt3_sb[:, 2, kc], tmp["g1"][:],
                                        0.375, None, mult)

            def lhsT(pt, kc, cs):
                cols = slice(cs * 128, (cs + 1) * 128)
                if pt in WR_SLOT:
                    return wr_sb[:, WR_SLOT[pt], kc, cols]
                return wt3_sb[:, pt - 3, kc, cols]

            pending_mout = None
            for slot, pt in enumerate(PT_ORDER):
                last = slot == len(PT_ORDER) - 1
                o = opool.tile([128, CS, UTH], mybir.dt.float16,
                               name=f"o_{pt}", tag="obuf")
                for cs in range(CS):
                    ps = psum_pool.tile([128, 512], mybir.dt.float32,
                                        name=f"ps_{pt}_{cs}", tag="ps")
                    for kc in range(KC):
                        nc.tensor.matmul(
                            ps[:],
                            lhsT(pt, kc, cs),
                            xt_sb[:, pt, kc, :],
                            start=(kc == 0),
                            stop=(kc == KC - 1),
                        )
                    nc.scalar.copy(o[:, cs], ps[:])
                    if last:
                        nc.scalar.dma_start(mout[pt, :, cs], o[:, cs])
                # Delay each point's output DMA by one slot so early output
                # packets queue behind the weight stream, not inside it.
                if pending_mout is not None:
                    ppt, po = pending_mout
                    nc.scalar.dma_start(mout[ppt], po[:])
                pending_mout = None if last else (pt, o)
            if pending_mout is not None:
                ppt, po = pending_mout
                nc.scalar.dma_start(mout[ppt], po[:])

    nc.compile()
    return nc


def _host_prep(X, W):
    """B^T input transform -> per-(b,th) xt chunks; fp16 weight basis."""
    Xpad = np.zeros((BSZ, T + 3, D), dtype=np.float32)
    Xpad[:, 3:] = X
    idx = np.arange(UT)[:, None] * 4 + np.arange(7)[None, :]
    xts = {}
    for b in range(BSZ):
        d = Xpad[b][idx]                                   # (UT, 7, D)
        xt_full = np.einsum('pj,ujc->puc', BT_M, d)        # (7, UT, D)
        for th in range(2):
            sl = xt_full[:, th * UTH:(th + 1) * UTH]       # (7, UTH, D)
            # -> [pt, kc-pair i, part p, j, u] with c = (2i+j)*128 + p
            arr = (sl.reshape(NPT, UTH, KC, 128)           # [pt,u,kc,p]
                   .transpose(0, 2, 3, 1)                  # [pt,kc,p,u]
                   .reshape(NPT, KC // 2, 2, 128, UTH)
                   .transpose(0, 1, 3, 2, 4))              # [pt,i,p,j,u]
            xts[(b, th)] = np.ascontiguousarray(arr, dtype=np.float16)

    W4 = W.reshape(KW, D, D)                               # [tap, co, cin]
    g = W4[::-1]                                           # g[j] = w[3-j]
    wrs = {}
    for ch in range(2):
        sl = g[:, ch * COH:(ch + 1) * COH, :]              # (4, COH, D)
        basis = np.stack([
            sl[0],                                         # g0   (pt 0)
            sl[3],                                         # g3   (pt inf)
            sl[0] + sl[1] + sl[2] + sl[3],                 # b1   (pt 1)
            sl[0] - sl[1] + sl[2] - sl[3],                 # b2   (pt -1)
        ])
        # -> [slot, kc, p, co] with cin = kc*128 + p
        arr = basis.transpose(0, 2, 1).reshape(KW, KC, 128, COH)
        wrs[ch] = np.ascontiguousarray(arr, dtype=np.float16)
    return xts, wrs


def kernel(X: np.ndarray, W: np.ndarray) -> np.ndarray:
    global _last_results, _nc_cache
    X = np.ascontiguousarray(X, dtype=np.float32)
    W = np.ascontiguousarray(W, dtype=np.float32)

    xts, wrs = _host_prep(X, W)

    if _nc_cache is None:
        _nc_cache = _build_nc()
    nc = _nc_cache

    # core c -> (batch, T-half, cout-half)
    def core_split(c):
        return c // 4, (c % 4) // 2, c % 2

    in_maps = []
    for c in range(NCORES):
        b, th, ch = core_split(c)
        in_maps.append({"xt": xts[(b, th)], "wr": wrs[ch]})
    _last_results = run_bass_kernel_spmd(nc, in_maps, core_ids=list(range(NCORES)))

    out = np.empty((BSZ, T, D), dtype=np.float32)
    for c in range(NCORES):
        b, th, ch = core_split(c)
        M = _last_results.results[c]["mout"].astype(np.float32)  # [7,128,CS,UTH]
        ob = np.einsum('qs,qpcu->uscp', A_M, M)                  # (UTH,4,CS,128)
        out[b, th * 2048:(th + 1) * 2048, ch * COH:(ch + 1) * COH] = \
            ob.reshape(UTH * 4, COH)
    return out


# revision 10
# speedup vs baseline: 1.0231x; 1.0231x over previous
"""Causal depthwise-conv self-attention kernel for Trainium2 (8 NeuronCores).

Math: out[b,t,d] = sum_i sum_k X[b,t-i,k] * W[i*D+d,k]   (i in 0..kW-1, zero for t<i)

Algorithm: Winograd F(4,4) over the time axis with points {0,1,-1,2,-2,1/2,inf}.
Each tile of 4 outputs needs 7 transform-point products instead of 16 tap-MACs,
cutting PE work to 7/16. Host applies the input transform B^T (7 points per
4-wide tile, exact fp32, cast fp16). The weight side uploads the fp16 BASIS
{g0, g3, b1=g0+g1+g2+g3, b2=g0-g1+g2-g3} (g = flipped taps): these are exactly
the G-transformed weights for points {0, inf, 1, -1}, so the first four
processed points need no device transform; points {2,-2,1/2} are derived from
the basis on the vector engine while those matmuls are still far away. The
device runs the 7 per-point (couts x cins) matmuls with fp32 PSUM
accumulation; host applies the 4x7 inverse transform A^T in fp32.

Sharding: 8 cores = 2 batches x 2 T-halves x 2 cout-halves. Per core:
X~ [7,128,8,512] fp16 (7.3 MB) + basis [4,8,128,512] fp16 (4.2 MB) in,
M [7,128,4,512] fp16 (3.7 MB) out, vs the 224-matmul (~48 us) PE floor.
DMA rings carry chunks in first-need order so the PE streams without stalls.
"""

import numpy as np

import concourse.bacc as bacc
import concourse.mybir as mybir
import concourse.tile as tile
from concourse.bass_utils import run_bass_kernel_spmd

# bass_utils imports antenv.axon_hooks when BASS_TRACE is set; that module is
# absent from this image. Provide a no-op stand-in so tracing degrades
# gracefully instead of crashing the run.
try:
    import antenv.axon_hooks  # noqa: F401
except ImportError:
    import sys
    import types

    import antenv

    _hooks = types.ModuleType("antenv.axon_hooks")
    _hooks._h = None
    _hooks.set_axon_ntff_profile_hook = lambda h: setattr(_hooks, "_h", h)
    _hooks.get_axon_ntff_profile_hook = lambda: _hooks._h
    sys.modules["antenv.axon_hooks"] = _hooks
    antenv.axon_hooks = _hooks

BSZ, T, D, KW = 2, 4096, 1024, 4
NCORES = 8
NPT = 7            # Winograd transform points for F(4,4)
UT = T // 4        # 4-wide output tiles = 1024
UTH = UT // 2      # tiles per T-half core = 512
KC = D // 128      # contraction chunks = 8
COH = D // 2       # output channels per cout-half core = 512
CS = COH // 128    # cout subtiles per core = 4
WARMUP_MMS = 10    # PE busy-burst during initial DMA (flips HAM to 8/8)
PT_ORDER = [0, 6, 1, 2, 3, 4, 5]   # basis points first, derived points last
# wr basis slot per point (pts 3,4,5 live in wt3_sb slots 0,1,2)
WR_SLOT = {0: 0, 6: 1, 1: 2, 2: 3}

_last_results = None   # test harness peeks at this for profiling info
_nc_cache = None       # compiled program reused across kernel() calls


def _build_transforms():
    points = [0.0, 1.0, -1.0, 2.0, -2.0, 0.5]   # 6 finite points + infinity
    V = np.zeros((7, 7))
    for k in range(7):
        for p, a in enumerate(points):
            V[k, p] = a ** k
    V[6, 6] = 1.0
    A = np.zeros((7, 4))
    for p, a in enumerate(points):
        for s in range(4):
            A[p, s] = a ** s
    A[6, 3] = 1.0
    BT = np.linalg.inv(V)
    return A.astype(np.float32), BT.astype(np.float32)


A_M, BT_M = _build_transforms()


def _build_nc():
    nc = bacc.Bacc(trn_type="TRN2", enable_partition_id=False)
    # p-major layouts: a kc-range slice has 1KB-per-kc contiguous partition
    # lines, so chunked transfers keep >=2KB DMA descriptors (1KB descriptors
    # measured ~3x slower per ring).
    xt = nc.dram_tensor("xt", [NPT, 128, KC, UTH], mybir.dt.float16,
                        kind="ExternalInput")
    wr = nc.dram_tensor("wr", [KW, 128, KC, COH], mybir.dt.float16,
                        kind="ExternalInput")
    mout = nc.dram_tensor("mout", [NPT, 128, CS, UTH], mybir.dt.float16,
                          kind="ExternalOutput")

    with tile.TileContext(nc) as tc:
        with (
            tc.tile_pool(name="xpool", bufs=1) as xpool,
            tc.tile_pool(name="wpool", bufs=1) as wpool,
            tc.tile_pool(name="tpool", bufs=2) as tpool,
            tc.tile_pool(name="opool", bufs=4) as opool,
            tc.tile_pool(name="psum", bufs=8, space="PSUM") as psum_pool,
        ):
            xt_sb = xpool.tile([128, NPT, KC, UTH], mybir.dt.float16)
            wr_sb = wpool.tile([128, KW, KC, COH], mybir.dt.float16)
            wt3_sb = wpool.tile([128, 3, KC, COH], mybir.dt.float16,
                                name="wt3")
            dummy = wpool.tile([128, 512], mybir.dt.float16, name="dummy")
            nc.gpsimd.memset(dummy[:].bitcast(mybir.dt.float32), 0.0)

            # DMA issue order == first-need order; the two HWDGE rings drain
            # round-robin at ~equal byte rate. Early chunks are fine-grained
            # (262KB) so the first matmuls start ASAP; later ones are coarse
            # for full descriptor efficiency.
            # sync:   xt in processing order; scalar: g0, g3, b1, b2 + mout.
            EARLY = [(0, 2), (2, 4), (4, 8)]
            def xt_chunks(pt, ranges):
                for lo, hi in ranges:
                    nc.sync.dma_start(xt_sb[:, pt, lo:hi, :],
                                      xt[pt, :, lo:hi, :])
            for pt in (0, 6, 1, 2):
                xt_chunks(pt, EARLY)
            for pt in (3, 4, 5):
                xt_chunks(pt, [(0, 8)])
            for j in range(KW):
                for lo, hi in EARLY:
                    nc.scalar.dma_start(wr_sb[:, j, lo:hi], wr[j, :, lo:hi])

            # HAM warmup: keep PE busy while the first DMAs land.
            ps_w = psum_pool.tile([128, 512], mybir.dt.float32,
                                  name="ps_warm", tag="ps")
            for w in range(WARMUP_MMS):
                nc.tensor.matmul(ps_w[:], dummy[:, :128], dummy[:],
                                 start=True, stop=True, skip_group_check=True)

            # Derive the weights of points {2,-2,1/2} from the basis
            # (vector engine only; consumed by the last three point groups):
            #   W3 = 3 b1 + b2 - 3 g0 + 6 g3
            #   W4 = b1 + 3 b2 - 3 g0 - 6 g3
            #   W5 = 0.375 (b1 - (1/3) b2 - g3 + 2 g0)
            mult, add = mybir.AluOpType.mult, mybir.AluOpType.add
            stt = nc.vector.scalar_tensor_tensor
            for kc in range(KC):
                g0, g3 = wr_sb[:, 0, kc], wr_sb[:, 1, kc]
                b1, b2 = wr_sb[:, 2, kc], wr_sb[:, 3, kc]
                tmp = {}
                for tg in ("c1", "c2", "d1", "d2", "f1", "f2", "g1"):
                    tmp[tg] = tpool.tile([128, COH], mybir.dt.float16,
                                         name=f"{tg}_{kc}", tag=tg)
                stt(tmp["c1"][:], b1, 3.0, b2, mult, add)
                stt(tmp["c2"][:], g0, -3.0, tmp["c1"][:], mult, add)
                stt(wt3_sb[:, 0, kc], g3, 6.0, tmp["c2"][:], mult, add)
                stt(tmp["d1"][:], b2, 3.0, b1, mult, add)
                stt(tmp["d2"][:], g0, -3.0, tmp["d1"][:], mult, add)
                stt(wt3_sb[:, 1, kc], g3, -6.0, tmp["d2"][:], mult, add)
                stt(tmp["f1"][:], b2, -1.0 / 3.0, b1, mult, add)
                nc.vector.tensor_sub(tmp["f2"][:], tmp["f1"][:], g3)
                stt(tmp["g1"][:], g0, 2.0, tmp["f2"][:], mult, add)
                nc.vector.tensor_scalar(wt3_sb[:, 2, kc], tmp["g1"][:],
                                        0.375, None, mult)

            def lhsT(pt, kc, cs):
                cols = slice(cs * 128, (cs + 1) * 128)
                if pt in WR_SLOT:
                    return wr_sb[:, WR_SLOT[pt], kc, cols]
                return wt3_sb[:, pt - 3, kc, cols]

            pending_mout = None
            for slot, pt in enumerate(PT_ORDER):
                last = slot == len(PT_ORDER) - 1
                o = opool.tile([128, CS, UTH], mybir.dt.float16,
                               name=f"o_{pt}", tag="obuf")
                for cs in range(CS):
                    ps = psum_pool.tile([128, 512], mybir.dt.float32,
                                        name=f"ps_{pt}_{cs}", tag="ps")
                    for kc in range(KC):
                        nc.tensor.matmul(
                            ps[:],
                            lhsT(pt, kc, cs),
                            xt_sb[:, pt, kc, :],
                            start=(kc == 0),
                            stop=(kc == KC - 1),
                        )
                    nc.scalar.copy(o[:, cs], ps[:])
                    if last:
                        nc.scalar.dma_start(mout[pt, :, cs], o[:, cs])
                # Delay each point's output DMA by one slot so early output
                # packets queue behind the weight stream, not inside it.
                if pending_mout is not None:
                    ppt, po = pending_mout
                    nc.scalar.dma_start(mout[ppt], po[:])
                pending_mout = None if last else (pt, o)
            if pending_mout is not None:
                ppt, po = pending_mout
                nc.scalar.dma_start(mout[ppt], po[:])

    nc.compile()
    return nc


def _host_prep(X, W):
    """B^T input transform -> per-(b,th) xt chunks; fp16 weight basis."""
    Xpad = np.zeros((BSZ, T + 3, D), dtype=np.float32)
    Xpad[:, 3:] = X
    idx = np.arange(UT)[:, None] * 4 + np.arange(7)[None, :]
    xts = {}
    for b in range(BSZ):
        d = Xpad[b][idx]                                   # (UT, 7, D)
        xt_full = np.einsum('pj,ujc->puc', BT_M, d)        # (7, UT, D)
        for th in range(2):
            sl = xt_full[:, th * UTH:(th + 1) * UTH]       # (7, UTH, D)
            # -> [pt, part p, kc, u] with c = kc*128 + p
            arr = sl.reshape(NPT, UTH, KC, 128).transpose(0, 3, 2, 1)
            xts[(b, th)] = np.ascontiguousarray(arr, dtype=np.float16)

    W4 = W.reshape(KW, D, D)                               # [tap, co, cin]
    g = W4[::-1]                                           # g[j] = w[3-j]
    wrs = {}
    for ch in range(2):
        sl = g[:, ch * COH:(ch + 1) * COH, :]              # (4, COH, D)
        basis = np.stack([
            sl[0],                                         # g0   (pt 0)
            sl[3],                                         # g3   (pt inf)
            sl[0] + sl[1] + sl[2] + sl[3],                 # b1   (pt 1)
            sl[0] - sl[1] + sl[2] - sl[3],                 # b2   (pt -1)
        ])
        # -> [slot, p, kc, co] with cin = kc*128 + p
        arr = (basis.transpose(0, 2, 1).reshape(KW, KC, 128, COH)
               .transpose(0, 2, 1, 3))
        wrs[ch] = np.ascontiguousarray(arr, dtype=np.float16)
    return xts, wrs


def kernel(X: np.ndarray, W: np.ndarray) -> np.ndarray:
    global _last_results, _nc_cache
    X = np.ascontiguousarray(X, dtype=np.float32)
    W = np.ascontiguousarray(W, dtype=np.float32)

    xts, wrs = _host_prep(X, W)

    if _nc_cache is None:
        _nc_cache = _build_nc()
    nc = _nc_cache

    # core c -> (batch, T-half, cout-half)
    def core_split(c):
        return c // 4, (c % 4) // 2, c % 2

    in_maps = []
    for c in range(NCORES):
        b, th, ch = core_split(c)
        in_maps.append({"xt": xts[(b, th)], "wr": wrs[ch]})
    _last_results = run_bass_kernel_spmd(nc, in_maps, core_ids=list(range(NCORES)))

    out = np.empty((BSZ, T, D), dtype=np.float32)
    for c in range(NCORES):
        b, th, ch = core_split(c)
        M = _last_results.results[c]["mout"].astype(np.float32)  # [7,128,CS,UTH]
        ob = np.einsum('qs,qpcu->uscp', A_M, M)                  # (UTH,4,CS,128)
        out[b, th * 2048:(th + 1) * 2048, ch * COH:(ch + 1) * COH] = \
            ob.reshape(UTH * 4, COH)
    return out


# revision 17
# speedup vs baseline: 1.1180x; 1.0928x over previous
"""Causal depthwise-conv self-attention kernel for Trainium2 (8 NeuronCores).

Math: out[b,t,d] = sum_i sum_k X[b,t-i,k] * W[i*D+d,k]   (i in 0..kW-1, zero for t<i)

Algorithm: Winograd F(4,4) over the time axis with points {0,1,-1,2,-2,1/2,inf}.
Each tile of 4 outputs needs 7 transform-point products instead of 16 tap-MACs,
cutting PE work to 7/16. Host applies the input transform B^T (7 points per
4-wide tile, exact fp32, cast fp16). The weight side uploads the fp16 BASIS
{g0, g3, b1=g0+g1+g2+g3, b2=g0-g1+g2-g3} (g = flipped taps): these are exactly
the G-transformed weights for points {0, inf, 1, -1}, so the first four
processed points need no device transform; points {2,-2,1/2} are derived from
the basis on the vector engine while those matmuls are still far away. The
device runs the 7 per-point (couts x cins) matmuls with fp32 PSUM
accumulation; host applies the 4x7 inverse transform A^T in fp32.

Sharding: 8 cores = 2 batches x 2 T-halves x 2 cout-halves. Per core:
X~ [7,128,8,512] fp16 (7.3 MB) + basis [4,8,128,512] fp16 (4.2 MB) in,
M [7,128,4,512] fp16 (3.7 MB) out, vs the 224-matmul (~48 us) PE floor.
DMA rings carry chunks in first-need order so the PE streams without stalls.
"""

import numpy as np

import concourse.bacc as bacc
import concourse.mybir as mybir
import concourse.tile as tile
from concourse.bass_utils import run_bass_kernel_spmd

# bass_utils imports antenv.axon_hooks when BASS_TRACE is set; that module is
# absent from this image. Provide a no-op stand-in so tracing degrades
# gracefully instead of crashing the run.
try:
    import antenv.axon_hooks  # noqa: F401
except ImportError:
    import sys
    import types

    import antenv

    _hooks = types.ModuleType("antenv.axon_hooks")
    _hooks._h = None
    _hooks.set_axon_ntff_profile_hook = lambda h: setattr(_hooks, "_h", h)
    _hooks.get_axon_ntff_profile_hook = lambda: _hooks._h
    sys.modules["antenv.axon_hooks"] = _hooks
    antenv.axon_hooks = _hooks

BSZ, T, D, KW = 2, 4096, 1024, 4
NCORES = 8
NPT = 7            # Winograd transform points for F(4,4)
UT = T // 4        # 4-wide output tiles = 1024
UTH = UT // 2      # tiles per T-half core = 512
KC = D // 128      # contraction chunks = 8
COH = D // 2       # output channels per cout-half core = 512
CS = COH // 128    # cout subtiles per core = 4
WARMUP_MMS = 10    # PE busy-burst during initial DMA (flips HAM to 8/8)
PT_ORDER = [0, 6, 1, 2, 3, 4, 5]   # uploaded points first, derived point last
# wr upload slot per point (pt 5 is derived on-device into wt1_sb)
WR_SLOT = {0: 0, 6: 1, 1: 2, 2: 3, 3: 4, 4: 5}

_last_results = None   # test harness peeks at this for profiling info
_nc_cache = None       # compiled program reused across kernel() calls


def _build_transforms():
    points = [0.0, 1.0, -1.0, 2.0, -2.0, 0.5]   # 6 finite points + infinity
    V = np.zeros((7, 7))
    for k in range(7):
        for p, a in enumerate(points):
            V[k, p] = a ** k
    V[6, 6] = 1.0
    A = np.zeros((7, 4))
    for p, a in enumerate(points):
        for s in range(4):
            A[p, s] = a ** s
    A[6, 3] = 1.0
    BT = np.linalg.inv(V)
    return A.astype(np.float32), BT.astype(np.float32)


A_M, BT_M = _build_transforms()


def _build_nc():
    nc = bacc.Bacc(trn_type="TRN2", enable_partition_id=False)
    # p-major layouts: a kc-range slice has 1KB-per-kc contiguous partition
    # lines, so chunked transfers keep >=2KB DMA descriptors (1KB descriptors
    # measured ~3x slower per ring).
    xt = nc.dram_tensor("xt", [NPT, 128, KC, UTH], mybir.dt.float16,
                        kind="ExternalInput")
    wr = nc.dram_tensor("wr", [6, 128, KC, COH], mybir.dt.float16,
                        kind="ExternalInput")
    mout = nc.dram_tensor("mout", [NPT, 128, CS, UTH], mybir.dt.float16,
                          kind="ExternalOutput")

    with tile.TileContext(nc) as tc:
        with (
            tc.tile_pool(name="xpool", bufs=1) as xpool,
            tc.tile_pool(name="wpool", bufs=1) as wpool,
            tc.tile_pool(name="tpool", bufs=2) as tpool,
            tc.tile_pool(name="opool", bufs=4) as opool,
            tc.tile_pool(name="psum", bufs=8, space="PSUM") as psum_pool,
        ):
            xt_sb = xpool.tile([128, NPT, KC, UTH], mybir.dt.float16)
            wr_sb = wpool.tile([128, 6, KC, COH], mybir.dt.float16)
            wt1_sb = wpool.tile([128, KC, COH], mybir.dt.float16,
                                name="wt1")
            dummy = wpool.tile([128, 512], mybir.dt.float16, name="dummy")
            nc.gpsimd.memset(dummy[:].bitcast(mybir.dt.float32), 0.0)

            # DMA issue order == first-need order; the two HWDGE rings drain
            # round-robin at ~equal byte rate. Early chunks are fine-grained
            # (262KB) so the first matmuls start ASAP; later ones are coarse
            # for full descriptor efficiency.
            # sync:   xt in processing order; scalar: g0, g3, b1, b2 + mout.
            EARLY = [(0, 2), (2, 4), (4, 8)]
            def xt_chunks(pt, ranges):
                for lo, hi in ranges:
                    nc.sync.dma_start(xt_sb[:, pt, lo:hi, :],
                                      xt[pt, :, lo:hi, :])
            for pt in (0, 6):
                xt_chunks(pt, EARLY)
            for pt in (1, 2, 3, 4, 5):
                xt_chunks(pt, [(0, 8)])
            for j in range(2):
                for lo, hi in EARLY:
                    nc.scalar.dma_start(wr_sb[:, j, lo:hi], wr[j, :, lo:hi])
            for j in range(2, 6):
                for lo, hi in ((0, 4), (4, 8)):
                    nc.scalar.dma_start(wr_sb[:, j, lo:hi], wr[j, :, lo:hi])

            # HAM warmup: keep PE busy while the first DMAs land.
            ps_w = psum_pool.tile([128, 512], mybir.dt.float32,
                                  name="ps_warm", tag="ps")
            for w in range(WARMUP_MMS):
                nc.tensor.matmul(ps_w[:], dummy[:, :128], dummy[:],
                                 start=True, stop=True, skip_group_check=True)

            # Derive the point-1/2 weights from the basis (vector engine,
            # consumed only by the final point group):
            #   W5 = 0.375 (2 g0 + b1 - (1/3) b2 - g3)
            mult, add = mybir.AluOpType.mult, mybir.AluOpType.add
            stt = nc.vector.scalar_tensor_tensor
            for kc in range(KC):
                g0, g3 = wr_sb[:, 0, kc], wr_sb[:, 1, kc]
                b1, b2 = wr_sb[:, 2, kc], wr_sb[:, 3, kc]
                tmp = {}
                for tg in ("f1", "f2", "g1"):
                    tmp[tg] = tpool.tile([128, COH], mybir.dt.float16,
                                         name=f"{tg}_{kc}", tag=tg)
                stt(tmp["f1"][:], b2, -1.0 / 3.0, b1, mult, add)
                nc.vector.tensor_sub(tmp["f2"][:], tmp["f1"][:], g3)
                stt(tmp["g1"][:], g0, 2.0, tmp["f2"][:], mult, add)
                nc.vector.tensor_scalar(wt1_sb[:, kc], tmp["g1"][:],
                                        0.375, None, mult)

            def lhsT(pt, kc, cs):
                cols = slice(cs * 128, (cs + 1) * 128)
                if pt == 5:
                    return wt1_sb[:, kc, cols]
                return wr_sb[:, WR_SLOT[pt], kc, cols]

            pending_mout = None
            for slot, pt in enumerate(PT_ORDER):
                tail = slot >= len(PT_ORDER) - 2
                o = opool.tile([128, CS, UTH], mybir.dt.float16,
                               name=f"o_{pt}", tag="obuf")
                for cs in range(CS):
                    ps = psum_pool.tile([128, 512], mybir.dt.float32,
                                        name=f"ps_{pt}_{cs}", tag="ps")
                    for kc in range(KC):
                        nc.tensor.matmul(
                            ps[:],
                            lhsT(pt, kc, cs),
                            xt_sb[:, pt, kc, :],
                            start=(kc == 0),
                            stop=(kc == KC - 1),
                        )
                    nc.scalar.copy(o[:, cs], ps[:])
                    if tail:
                        nc.scalar.dma_start(mout[pt, :, cs], o[:, cs])
                # Delay each point's output DMA by one slot so early output
                # packets queue behind the weight stream, not inside it; the
                # last two points stream out per-cs to shorten the tail.
                if pending_mout is not None:
                    ppt, po = pending_mout
                    nc.scalar.dma_start(mout[ppt], po[:])
                pending_mout = None if tail else (pt, o)
            if pending_mout is not None:
                ppt, po = pending_mout
                nc.scalar.dma_start(mout[ppt], po[:])

    nc.compile()
    return nc


def _host_prep(X, W):
    """B^T input transform -> per-(b,th) xt chunks; fp16 weight basis."""
    Xpad = np.zeros((BSZ, T + 3, D), dtype=np.float32)
    Xpad[:, 3:] = X
    idx = np.arange(UT)[:, None] * 4 + np.arange(7)[None, :]
    xts = {}
    for b in range(BSZ):
        d = Xpad[b][idx]                                   # (UT, 7, D)
        xt_full = np.einsum('pj,ujc->puc', BT_M, d)        # (7, UT, D)
        for th in range(2):
            sl = xt_full[:, th * UTH:(th + 1) * UTH]       # (7, UTH, D)
            # -> [pt, part p, kc, u] with c = kc*128 + p
            arr = sl.reshape(NPT, UTH, KC, 128).transpose(0, 3, 2, 1)
            xts[(b, th)] = np.ascontiguousarray(arr, dtype=np.float16)

    W4 = W.reshape(KW, D, D)                               # [tap, co, cin]
    g = W4[::-1]                                           # g[j] = w[3-j]
    wrs = {}
    for ch in range(2):
        sl = g[:, ch * COH:(ch + 1) * COH, :]              # (4, COH, D)
        basis = np.stack([
            sl[0],                                         # g0      (pt 0)
            sl[3],                                         # g3      (pt inf)
            sl[0] + sl[1] + sl[2] + sl[3],                 # b1      (pt 1)
            sl[0] - sl[1] + sl[2] - sl[3],                 # b2      (pt -1)
            sl[0] + 2 * sl[1] + 4 * sl[2] + 8 * sl[3],     # W3      (pt 2)
            sl[0] - 2 * sl[1] + 4 * sl[2] - 8 * sl[3],     # W4      (pt -2)
        ])
        # -> [slot, p, kc, co] with cin = kc*128 + p
        arr = (basis.transpose(0, 2, 1).reshape(6, KC, 128, COH)
               .transpose(0, 2, 1, 3))
        wrs[ch] = np.ascontiguousarray(arr, dtype=np.float16)
    return xts, wrs


def kernel(X: np.ndarray, W: np.ndarray) -> np.ndarray:
    global _last_results, _nc_cache
    X = np.ascontiguousarray(X, dtype=np.float32)
    W = np.ascontiguousarray(W, dtype=np.float32)

    xts, wrs = _host_prep(X, W)

    if _nc_cache is None:
        _nc_cache = _build_nc()
    nc = _nc_cache

    # core c -> (batch, T-half, cout-half)
    def core_split(c):
        return c // 4, (c % 4) // 2, c % 2

    in_maps = []
    for c in range(NCORES):
        b, th, ch = core_split(c)
        in_maps.append({"xt": xts[(b, th)], "wr": wrs[ch]})
    _last_results = run_bass_kernel_spmd(nc, in_maps, core_ids=list(range(NCORES)))

    out = np.empty((BSZ, T, D), dtype=np.float32)
    for c in range(NCORES):
        b, th, ch = core_split(c)
        M = _last_results.results[c]["mout"].astype(np.float32)  # [7,128,CS,UTH]
        ob = np.einsum('qs,qpcu->uscp', A_M, M)                  # (UTH,4,CS,128)
        out[b, th * 2048:(th + 1) * 2048, ch * COH:(ch + 1) * COH] = \
            ob.reshape(UTH * 4, COH)
    return out


# revision 18
# speedup vs baseline: 1.2156x; 1.0874x over previous
"""Causal depthwise-conv self-attention kernel for Trainium2 (8 NeuronCores).

Math: out[b,t,d] = sum_i sum_k X[b,t-i,k] * W[i*D+d,k]   (i in 0..kW-1, zero for t<i)

Algorithm: Winograd F(4,4) over the time axis with points {0,1,-1,2,-2,1/2,inf}.
Each tile of 4 outputs needs 7 transform-point products instead of 16 tap-MACs,
cutting PE work to 7/16. Host applies the input transform B^T (7 points per
4-wide tile, exact fp32, cast fp16). The weight side uploads the fp16 BASIS
{g0, g3, b1=g0+g1+g2+g3, b2=g0-g1+g2-g3} (g = flipped taps): these are exactly
the G-transformed weights for points {0, inf, 1, -1}, so the first four
processed points need no device transform; points {2,-2,1/2} are derived from
the basis on the vector engine while those matmuls are still far away. The
device runs the 7 per-point (couts x cins) matmuls with fp32 PSUM
accumulation; host applies the 4x7 inverse transform A^T in fp32.

Sharding: 8 cores = 2 batches x 2 T-halves x 2 cout-halves. Per core:
X~ [7,128,8,512] fp16 (7.3 MB) + basis [4,8,128,512] fp16 (4.2 MB) in,
M [7,128,4,512] fp16 (3.7 MB) out, vs the 224-matmul (~48 us) PE floor.
DMA rings carry chunks in first-need order so the PE streams without stalls.
"""

import numpy as np

import concourse.bacc as bacc
import concourse.mybir as mybir
import concourse.tile as tile
from concourse.bass_utils import run_bass_kernel_spmd

# bass_utils imports antenv.axon_hooks when BASS_TRACE is set; that module is
# absent from this image. Provide a no-op stand-in so tracing degrades
# gracefully instead of crashing the run.
try:
    import antenv.axon_hooks  # noqa: F401
except ImportError:
    import sys
    import types

    import antenv

    _hooks = types.ModuleType("antenv.axon_hooks")
    _hooks._h = None
    _hooks.set_axon_ntff_profile_hook = lambda h: setattr(_hooks, "_h", h)
    _hooks.get_axon_ntff_profile_hook = lambda: _hooks._h
    sys.modules["antenv.axon_hooks"] = _hooks
    antenv.axon_hooks = _hooks

BSZ, T, D, KW = 2, 4096, 1024, 4
NCORES = 8
NPT = 7            # Winograd transform points for F(4,4)
UT = T // 4        # 4-wide output tiles = 1024
UTH = UT // 2      # tiles per T-half core = 512
KC = D // 128      # contraction chunks = 8
COH = D // 2       # output channels per cout-half core = 512
CS = COH // 128    # cout subtiles per core = 4
WARMUP_MMS = 22    # PE busy-burst until first operands land (keeps HAM at 8/8)
PT_ORDER = [0, 6, 1, 2, 3, 4, 5]   # uploaded points first, derived point last
# wr upload slot per point (pt 5 is derived on-device into wt1_sb)
WR_SLOT = {0: 0, 6: 1, 1: 2, 2: 3, 3: 4, 4: 5}

_last_results = None   # test harness peeks at this for profiling info
_nc_cache = None       # compiled program reused across kernel() calls


def _build_transforms():
    points = [0.0, 1.0, -1.0, 2.0, -2.0, 0.5]   # 6 finite points + infinity
    V = np.zeros((7, 7))
    for k in range(7):
        for p, a in enumerate(points):
            V[k, p] = a ** k
    V[6, 6] = 1.0
    A = np.zeros((7, 4))
    for p, a in enumerate(points):
        for s in range(4):
            A[p, s] = a ** s
    A[6, 3] = 1.0
    BT = np.linalg.inv(V)
    return A.astype(np.float32), BT.astype(np.float32)


A_M, BT_M = _build_transforms()


def _build_nc():
    nc = bacc.Bacc(trn_type="TRN2", enable_partition_id=False)
    # p-major layouts: a kc-range slice has 1KB-per-kc contiguous partition
    # lines, so chunked transfers keep >=2KB DMA descriptors (1KB descriptors
    # measured ~3x slower per ring).
    xt = nc.dram_tensor("xt", [NPT, 128, KC, UTH], mybir.dt.float16,
                        kind="ExternalInput")
    wr = nc.dram_tensor("wr", [6, 128, KC, COH], mybir.dt.float16,
                        kind="ExternalInput")
    mout = nc.dram_tensor("mout", [NPT, 128, CS, UTH], mybir.dt.float16,
                          kind="ExternalOutput")

    with tile.TileContext(nc) as tc:
        with (
            tc.tile_pool(name="xpool", bufs=1) as xpool,
            tc.tile_pool(name="wpool", bufs=1) as wpool,
            tc.tile_pool(name="tpool", bufs=2) as tpool,
            tc.tile_pool(name="opool", bufs=4) as opool,
            tc.tile_pool(name="psum", bufs=8, space="PSUM") as psum_pool,
        ):
            xt_sb = xpool.tile([128, NPT, KC, UTH], mybir.dt.float16)
            wr_sb = wpool.tile([128, 6, KC, COH], mybir.dt.float16)
            wt1_sb = wpool.tile([128, KC, COH], mybir.dt.float16,
                                name="wt1")
            dummy = wpool.tile([128, 512], mybir.dt.float16, name="dummy")
            nc.gpsimd.memset(dummy[:].bitcast(mybir.dt.float32), 0.0)

            # DMA issue order == first-need order; the two HWDGE rings drain
            # round-robin at ~equal byte rate. Early chunks are fine-grained
            # (262KB) so the first matmuls start ASAP; later ones are coarse
            # for full descriptor efficiency.
            # sync:   xt in processing order; scalar: g0, g3, b1, b2 + mout.
            EARLY = [(0, 2), (2, 4), (4, 8)]
            def xt_chunks(pt, ranges):
                for lo, hi in ranges:
                    nc.sync.dma_start(xt_sb[:, pt, lo:hi, :],
                                      xt[pt, :, lo:hi, :])
            for pt in (0, 6):
                xt_chunks(pt, EARLY)
            for pt in (1, 2, 3, 4, 5):
                xt_chunks(pt, [(0, 8)])
            for j in range(2):
                for lo, hi in EARLY:
                    nc.scalar.dma_start(wr_sb[:, j, lo:hi], wr[j, :, lo:hi])
            for j in range(2, 6):
                for lo, hi in ((0, 4), (4, 8)):
                    nc.scalar.dma_start(wr_sb[:, j, lo:hi], wr[j, :, lo:hi])

            # HAM warmup: keep PE busy while the first DMAs land.
            ps_w = psum_pool.tile([128, 512], mybir.dt.float32,
                                  name="ps_warm", tag="ps")
            for w in range(WARMUP_MMS):
                nc.tensor.matmul(ps_w[:], dummy[:, :128], dummy[:],
                                 start=True, stop=True, skip_group_check=True)

            # Derive the point-1/2 weights from the basis (vector engine,
            # consumed only by the final point group):
            #   W5 = 0.375 (2 g0 + b1 - (1/3) b2 - g3)
            mult, add = mybir.AluOpType.mult, mybir.AluOpType.add
            stt = nc.vector.scalar_tensor_tensor
            for kc in range(KC):
                g0, g3 = wr_sb[:, 0, kc], wr_sb[:, 1, kc]
                b1, b2 = wr_sb[:, 2, kc], wr_sb[:, 3, kc]
                tmp = {}
                for tg in ("f1", "f2", "g1"):
                    tmp[tg] = tpool.tile([128, COH], mybir.dt.float16,
                                         name=f"{tg}_{kc}", tag=tg)
                stt(tmp["f1"][:], b2, -1.0 / 3.0, b1, mult, add)
                nc.vector.tensor_sub(tmp["f2"][:], tmp["f1"][:], g3)
                stt(tmp["g1"][:], g0, 2.0, tmp["f2"][:], mult, add)
                nc.vector.tensor_scalar(wt1_sb[:, kc], tmp["g1"][:],
                                        0.375, None, mult)

            def lhsT(pt, kc, cs):
                cols = slice(cs * 128, (cs + 1) * 128)
                if pt == 5:
                    return wt1_sb[:, kc, cols]
                return wr_sb[:, WR_SLOT[pt], kc, cols]

            pending_mout = None
            for slot, pt in enumerate(PT_ORDER):
                tail = slot >= len(PT_ORDER) - 2
                o = opool.tile([128, CS, UTH], mybir.dt.float16,
                               name=f"o_{pt}", tag="obuf")
                for cs in range(CS):
                    ps = psum_pool.tile([128, 512], mybir.dt.float32,
                                        name=f"ps_{pt}_{cs}", tag="ps")
                    for kc in range(KC):
                        nc.tensor.matmul(
                            ps[:],
                            lhsT(pt, kc, cs),
                            xt_sb[:, pt, kc, :],
                            start=(kc == 0),
                            stop=(kc == KC - 1),
                        )
                    nc.scalar.copy(o[:, cs], ps[:])
                    if tail:
                        nc.sync.dma_start(mout[pt, :, cs], o[:, cs])
                # Delay each point's output DMA by one slot so early output
                # packets queue behind the weight stream, not inside it; the
                # last two points stream out per-cs to shorten the tail.
                if pending_mout is not None:
                    ppt, po = pending_mout
                    nc.sync.dma_start(mout[ppt], po[:])
                pending_mout = None if tail else (pt, o)
            if pending_mout is not None:
                ppt, po = pending_mout
                nc.sync.dma_start(mout[ppt], po[:])

    nc.compile()
    return nc


def _host_prep(X, W):
    """B^T input transform -> per-(b,th) xt chunks; fp16 weight basis."""
    Xpad = np.zeros((BSZ, T + 3, D), dtype=np.float32)
    Xpad[:, 3:] = X
    idx = np.arange(UT)[:, None] * 4 + np.arange(7)[None, :]
    xts = {}
    for b in range(BSZ):
        d = Xpad[b][idx]                                   # (UT, 7, D)
        xt_full = np.einsum('pj,ujc->puc', BT_M, d)        # (7, UT, D)
        for th in range(2):
            sl = xt_full[:, th * UTH:(th + 1) * UTH]       # (7, UTH, D)
            # -> [pt, part p, kc, u] with c = kc*128 + p
            arr = sl.reshape(NPT, UTH, KC, 128).transpose(0, 3, 2, 1)
            xts[(b, th)] = np.ascontiguousarray(arr, dtype=np.float16)

    W4 = W.reshape(KW, D, D)                               # [tap, co, cin]
    g = W4[::-1]                                           # g[j] = w[3-j]
    wrs = {}
    for ch in range(2):
        sl = g[:, ch * COH:(ch + 1) * COH, :]              # (4, COH, D)
        basis = np.stack([
            sl[0],                                         # g0      (pt 0)
            sl[3],                                         # g3      (pt inf)
            sl[0] + sl[1] + sl[2] + sl[3],                 # b1      (pt 1)
            sl[0] - sl[1] + sl[2] - sl[3],                 # b2      (pt -1)
            sl[0] + 2 * sl[1] + 4 * sl[2] + 8 * sl[3],     # W3      (pt 2)
            sl[0] - 2 * sl[1] + 4 * sl[2] - 8 * sl[3],     # W4      (pt -2)
        ])
        # -> [slot, p, kc, co] with cin = kc*128 + p
        arr = (basis.transpose(0, 2, 1).reshape(6, KC, 128, COH)
               .transpose(0, 2, 1, 3))
        wrs[ch] = np.ascontiguousarray(arr, dtype=np.float16)
    return xts, wrs


def kernel(X: np.ndarray, W: np.ndarray) -> np.ndarray:
    global _last_results, _nc_cache
    X = np.ascontiguousarray(X, dtype=np.float32)
    W = np.ascontiguousarray(W, dtype=np.float32)

    xts, wrs = _host_prep(X, W)

    if _nc_cache is None:
        _nc_cache = _build_nc()
    nc = _nc_cache

    # core c -> (batch, T-half, cout-half)
    def core_split(c):
        return c // 4, (c % 4) // 2, c % 2

    in_maps = []
    for c in range(NCORES):
        b, th, ch = core_split(c)
        in_maps.append({"xt": xts[(b, th)], "wr": wrs[ch]})
    _last_results = run_bass_kernel_spmd(nc, in_maps, core_ids=list(range(NCORES)))

    out = np.empty((BSZ, T, D), dtype=np.float32)
    for c in range(NCORES):
        b, th, ch = core_split(c)
        M = _last_results.results[c]["mout"].astype(np.float32)  # [7,128,CS,UTH]
        ob = np.einsum('qs,qpcu->uscp', A_M, M)                  # (UTH,4,CS,128)
        out[b, th * 2048:(th + 1) * 2048, ch * COH:(ch + 1) * COH] = \
            ob.reshape(UTH * 4, COH)
    return out
